# revision 17
# baseline (speedup 1.0000x reference)
"""Trainium2 Bass kernel for nn_Attention (B=4, N=1024, DIM=1024, H=16).

Sharding: 8 cores = 4 batches x 2 query-halves of 512 rows each. No
collectives - each core recomputes its batch's K/V projections.

Key design (cost model: matmul cost = moving rows x 0.4167ns x cyc/row;
fp8e4m3 DoubleRow = 0.5 cyc/row with 2x128 contraction = 4x bf16):
  - K/V projections and a scores-only Q projection run in fp8 DoubleRow.
    Weights are pre-scaled x16 (Wq/Wk) so fp8 quantization stays in the
    normal range; the 1/(32*16*16) total score scale folds into the exp.
  - Scores use a DMA-shuffled [32, 2, *] layout (dims interleaved into
    partition+slot) so the 64-dim head contraction runs as one DoubleRow
    matmul; A*V uses slot = key-tile pairs with fp8 es written directly
    by the exp.
  - The residual path stays bf16: precise Qp projection (direct [q, d]
    layout, no transposes) and bf16 fc_o. Attention output is ~30x
    smaller than Qp, so fp8 noise there is harmless.
  - LayerNorm rstd = exp(-0.5*ln(var+eps)) keeps ACT on the exp/ln
    table set (3 table loads total: exp/ln -> gelu -> exp/ln).
  - Masking: masked K rows are zeroed via the km scalar in the V
    evacuation and excluded from the softmax denominator (65th V
    column = km01); masked Q rows are zeroed by the final qmask scale.
"""

import numpy as np
import ml_dtypes
from contextlib import ExitStack

import concourse.bass as bass
import concourse.bacc as bacc
import concourse.mybir as mybir
import concourse.tile as tile
from concourse.bass_utils import run_bass_kernel_spmd
from concourse.masks import make_identity

FP = mybir.dt.float32
BF = mybir.dt.bfloat16
F8 = mybir.dt.float8e4
AF = mybir.ActivationFunctionType
ALU = mybir.AluOpType
DRM = mybir.MatmulPerfMode.DoubleRow

DIM = 1024
H = 16
DH = 64
B = 4
N = 1024          # keys per batch
NQ = 512          # queries per core
P = 128
NDT = DIM // P    # 8 feature tiles
NKT = N // P      # 8 key tiles
NQT = NQ // P     # 4 query tiles
NG = 4            # DoubleRow contraction groups (4 x 2 x 128 = 1024)
EPS = 1e-5
ESCALE = 1.0 / 8192.0   # 1/(sqrt(DIM) * 16 * 16)

_CACHED_NC = None


def build_nc():
    nc = bacc.Bacc(None, target_bir_lowering=False, debug=True)
    qt8 = nc.declare_dram_parameter("qt8", [P, NG, 2, NQ], F8, isOutput=False)
    wq8 = nc.declare_dram_parameter("wq8", [P, NG, 2, DIM], F8, isOutput=False)
    kt8 = nc.declare_dram_parameter("kt8", [P, NG, 2, N], F8, isOutput=False)
    wk8 = nc.declare_dram_parameter("wk8", [P, NG, 2, DIM], F8, isOutput=False)
    wv8 = nc.declare_dram_parameter("wv8", [P, NG, 2, DIM], F8, isOutput=False)
    vt8 = nc.declare_dram_parameter("vt8", [P, NG, 2, N], F8, isOutput=False)
    qtb = nc.declare_dram_parameter("qtb", [P, NDT, NQ], BF, isOutput=False)
    wqb = nc.declare_dram_parameter("wqb", [P, NDT, DIM], BF, isOutput=False)
    wo = nc.declare_dram_parameter("wo", [P, NDT, DIM], BF, isOutput=False)
    # maskd: cols 0..7 = km01 key-tile columns, 8..11 = qm01 query-tile cols
    maskd = nc.declare_dram_parameter("maskd", [P, NKT + NQT], FP, isOutput=False)
    out = nc.declare_dram_parameter("out", [NQ, DIM], BF, isOutput=True)

    with ExitStack() as ctx:
        tc = ctx.enter_context(tile.TileContext(nc))
        persist = ctx.enter_context(tc.tile_pool(name="persist", bufs=1))

        # ---- persistent SBUF tiles ----
        mask_sb = persist.tile([P, NKT + NQT], FP, tag="mask", name="mask_sb")
        qt8_sb = persist.tile([P, NG, 2, NQ], F8, tag="qt8", name="qt8_sb")
        wq8_sb = persist.tile([P, NG, 2, DIM], F8, tag="wq8", name="wq8_sb")
        kt8_sb = persist.tile([P, NG, 2, N], F8, tag="kt8", name="kt8_sb")
        wk8_sb = persist.tile([P, NG, 2, DIM], F8, tag="wk8", name="wk8_sb")
        wv8_sb = persist.tile([P, NG, 2, DIM], F8, tag="wv8", name="wv8_sb")
        vt8_sb = persist.tile([P, NG, 2, N], F8, tag="vt8", name="vt8_sb")
        qtb_sb = persist.tile([P, NDT, NQ], BF, tag="qtb", name="qtb_sb")
        wqb_sb = persist.tile([P, NDT, DIM], BF, tag="wqb", name="wqb_sb")
        wo_sb = persist.tile([P, NDT, DIM], BF, tag="wo", name="wo_sb")
        Q8pT = [persist.tile([P, NQ], F8, tag=f"q8pt{i}", name=f"q8pt{i}")
                for i in range(NDT)]
        KpT8 = [persist.tile([P, N], F8, tag=f"kpt8{i}", name=f"kpt8{i}")
                for i in range(NDT)]
        QDR = [persist.tile([64, 2, NQ], F8, tag=f"qdr{i}", name=f"qdr{i}")
               for i in range(NDT)]
        KDR = [persist.tile([64, 2, N], F8, tag=f"kdr{i}", name=f"kdr{i}")
               for i in range(NDT)]
        # VDR[jp]: slot t = key-tile 2jp+t; per head 66 cols (64 dims,
        # col 64 = km01 denominator column, col 65 pad)
        VDR = [persist.tile([P, 2, H, DH + 2], F8, tag=f"vdr{j}", name=f"vdr{j}")
               for j in range(NKT // 2)]
        Qp = [persist.tile([P, DIM], BF, tag=f"qp{t}", name=f"qp{t}")
              for t in range(NQT)]
        Ob = persist.tile([P, NQT, DIM], BF, tag="ob", name="ob")
        O1 = [persist.tile([P, DIM], BF, tag=f"o1_{t}", name=f"o1_{t}")
              for t in range(NQT)]
        OTb = persist.tile([P, NDT, NQ], BF, tag="otb", name="otb")
        identb = persist.tile([P, P], BF, tag="identb", name="identb")
        make_identity(nc, identb)
        eps_sb = persist.tile([P, 1], FP, tag="eps", name="eps_sb")
        nc.vector.memset(eps_sb, EPS)

        km = mask_sb[:, 0:NKT]
        qm = mask_sb[:, NKT:NKT + NQT]

        # ---- input DMAs (SP, issue order = priority order) ----
        nc.sync.dma_start(out=mask_sb, in_=maskd[:, :])
        nc.sync.dma_start(out=qt8_sb, in_=qt8[:, :, :, :])
        nc.sync.dma_start(out=wq8_sb, in_=wq8[:, :, :, :])
        nc.sync.dma_start(out=kt8_sb, in_=kt8[:, :, :, :])
        nc.sync.dma_start(out=wk8_sb[:, :, :, 0:512], in_=wk8[:, :, :, 0:512])
        nc.sync.dma_start(out=wk8_sb[:, :, :, 512:1024], in_=wk8[:, :, :, 512:1024])
        # wv8/vt8/qtb/wqb/wo are issued AFTER the QDR/KDR shuffles below:
        # they are not needed until later, and the serial DMA device would
        # otherwise delay the latency-critical shuffles (first scores).

        # ---- phase 1a: scores-Q projection (fp8 DR) ----
        with tc.tile_pool(name="q8ps", bufs=2, space="PSUM") as q8ps:
            for i in range(NDT):
                ps = q8ps.tile([P, NQ], FP, tag="ps", name=f"q8ps{i}")
                for g in range(NG):
                    nc.tensor.matmul(ps, wq8_sb[:, g, :, P * i:P * (i + 1)],
                                     qt8_sb[:, g, :, :],
                                     start=(g == 0), stop=(g == NG - 1),
                                     perf_mode=DRM)
                nc.vector.tensor_copy(Q8pT[i], ps)

        # ---- phase 1b: K projection (fp8 DR) ----
        with tc.tile_pool(name="kps", bufs=2, space="PSUM") as kps:
            for i in range(NDT):
                ps = kps.tile([P, 2, 512], FP, tag="ps", name=f"kps{i}")
                for c in range(2):
                    for g in range(NG):
                        nc.tensor.matmul(ps[:, c, :],
                                         wk8_sb[:, g, :, P * i:P * (i + 1)],
                                         kt8_sb[:, g, :, 512 * c:512 * c + 512],
                                         start=(g == 0), stop=(g == NG - 1),
                                         perf_mode=DRM)
                nc.vector.tensor_copy(KpT8[i], ps.rearrange("p c n -> p (c n)"))

        # ---- DR-layout shuffles (partition p -> (p//2, slot p%2)) ----
        for i in range(NDT):
            nc.sync.dma_start(out=QDR[i], in_=Q8pT[i][:, :])
        for i in range(NDT):
            nc.sync.dma_start(out=KDR[i], in_=KpT8[i][:, :])
        nc.sync.dma_start(out=wv8_sb, in_=wv8[:, :, :, :])
        nc.sync.dma_start(out=vt8_sb, in_=vt8[:, :, :, :])
        nc.sync.dma_start(out=qtb_sb, in_=qtb[:, :, :])
        nc.sync.dma_start(out=wqb_sb[:, 0:4], in_=wqb[:, 0:4, :])
        nc.sync.dma_start(out=wqb_sb[:, 4:8], in_=wqb[:, 4:8, :])
        nc.sync.dma_start(out=wo_sb[:, 0:4], in_=wo[:, 0:4, :])
        nc.sync.dma_start(out=wo_sb[:, 4:8], in_=wo[:, 4:8, :])

        # ---- phase 2: attention (+ V proj and Qp proj interleaved) ----
        # PSUM: spp 3x2 banks (scores ring; V/Qp proj reuse its slots) +
        # avp 1 + tpp 1 = 8.
        es_tiles = {}
        LAG = 3

        # r1 tiles + LN1 stats persist so the heads-0..7 half can be
        # computed mid-stream (head h owns output columns 64h..64h+63).
        r1s = [persist.tile([P, DIM], BF, tag=f"r1_{t}", name=f"r1_{t}")
               for t in range(NQT)]
        st1 = [persist.tile([P, 2, 6], FP, tag=f"st1_{t}", name=f"st1_{t}")
               for t in range(NQT)]

        with tc.tile_pool(name="spp", bufs=3, space="PSUM") as spp, \
             tc.tile_pool(name="avp", bufs=1, space="PSUM") as avp, \
             tc.tile_pool(name="tpp", bufs=1, space="PSUM") as tpp, \
             tc.tile_pool(name="esp", bufs=4) as esp, \
             tc.tile_pool(name="p2sb", bufs=2) as p2sb:

            def scores_head(h):
                i, g = h // 2, 32 * (h % 2)
                for jp in range(NKT // 2):
                    sp = spp.tile([P, 2, NQ], FP, tag="sp", name=f"sp{h}_{jp}")
                    for s in range(2):
                        k = 2 * jp + s
                        nc.tensor.matmul(sp[:, s, :],
                                         KDR[i][g:g + 32, :, P * k:P * (k + 1)],
                                         QDR[i][g:g + 32, :, :],
                                         start=True, stop=True, perf_mode=DRM)
                    es = esp.tile([P, 2, NQ], F8, tag=f"es{jp}", name=f"es{h}_{jp}")
                    nc.scalar.activation(out=es, in_=sp, func=AF.Exp, scale=ESCALE)
                    es_tiles[(h, jp)] = es

            def av_head(h):
                av = avp.tile([DH + 1, NQ], FP, tag="av", name=f"av{h}")
                for jp in range(NKT // 2):
                    nc.tensor.matmul(av, VDR[jp][:, :, h, 0:DH + 1],
                                     es_tiles.pop((h, jp)),
                                     start=(jp == 0), stop=(jp == NKT // 2 - 1),
                                     perf_mode=DRM)
                avsb = p2sb.tile([DH + 1, NQ], BF, tag="avsb", name=f"avsb{h}")
                nc.vector.tensor_copy(avsb, av)
                tpg = tpp.tile([P, NQT, DH + 2], BF, tag="tpg", name=f"tpg{h}")
                for t in range(NQT):
                    nc.tensor.matmul(tpg[:, t, 0:DH + 1],
                                     avsb[:, P * t:P * (t + 1)],
                                     identb[0:DH + 1, 0:DH + 1],
                                     is_transpose=True,
                                     start=(t == 0), stop=(t == NQT - 1))
                osb = p2sb.tile([P, NQT, DH + 2], BF, tag="osb", name=f"osb{h}")
                nc.vector.tensor_copy(osb[:, :, 0:DH + 1], tpg[:, :, 0:DH + 1])
                dr = p2sb.tile([P, NQT, 1], BF, tag="dr", name=f"dr{h}")
                with nc.allow_low_precision(
                        reason="denom ~512, bf16 recip err 0.4% on a term 30x "
                               "smaller than the residual"):
                    nc.vector.reciprocal(out=dr, in_=osb[:, :, DH:DH + 1])
                nc.vector.tensor_mul(Ob[:, :, DH * h:DH * (h + 1)],
                                     osb[:, :, 0:DH],
                                     dr.to_broadcast((P, NQT, DH)))

            def v_proj():
                for i in range(NKT):
                    ps = spp.tile([P, 2, NQ], FP, tag="sp", name=f"vps{i}")
                    for c in range(2):
                        for g in range(NG):
                            nc.tensor.matmul(ps[:, c, :],
                                             vt8_sb[:, g, :, P * i:P * (i + 1)],
                                             wv8_sb[:, g, :, 512 * c:512 * (c + 1)],
                                             start=(g == 0), stop=(g == NG - 1),
                                             perf_mode=DRM)
                    nc.vector.tensor_scalar_mul(
                        out=VDR[i // 2][:, i % 2, :, 0:DH],
                        in0=ps.rearrange("p c (h d) -> p (c h) d", h=8),
                        scalar1=km[:, i:i + 1])
                # denominator columns (km01, excluded keys contribute 0)
                for jp in range(NKT // 2):
                    nc.vector.tensor_copy(
                        VDR[jp][:, :, :, DH:DH + 1],
                        km[:, 2 * jp:2 * jp + 2].to_broadcast((P, 2, H, 1)))

            def qp_proj_part(t):
                ps = spp.tile([P, 2, NQ], FP, tag="sp", name=f"qpp{t}")
                for c in range(2):
                    for j in range(NDT):
                        nc.tensor.matmul(ps[:, c, :],
                                         qtb_sb[:, j, P * t:P * (t + 1)],
                                         wqb_sb[:, j, 512 * c:512 * (c + 1)],
                                         start=(j == 0), stop=(j == NDT - 1))
                nc.vector.tensor_copy(Qp[t], ps.rearrange("p c n -> p (c n)"))

            def r1_half(s):
                lo, hi = 512 * s, 512 * (s + 1)
                for t in range(NQT):
                    nc.vector.tensor_add(r1s[t][:, lo:hi], Qp[t][:, lo:hi],
                                         Ob[:, t, lo:hi])
                    nc.vector.bn_stats(out=st1[t][:, s, :], in_=r1s[t][:, lo:hi])

            for h in range(H):
                scores_head(h)
                if h == 3:
                    v_proj()
                if h >= LAG:
                    av_head(h - LAG)
                if 6 <= h <= 9:
                    qp_proj_part(h - 6)
                if h == 11:
                    r1_half(0)  # needs heads 0..7 = av_head(7) done (h==10)
            for h in range(H - LAG, H):
                av_head(h)

        # ---- phase 3: residual + LN1 + fc_o + GELU + LN2 ----
        with tc.tile_pool(name="p3", bufs=1) as p3, \
             tc.tile_pool(name="p3s", bufs=2) as p3s, \
             tc.tile_pool(name="tpp3", bufs=2, space="PSUM") as tpp3, \
             tc.tile_pool(name="fps", bufs=2, space="PSUM") as fps:

            def ln_rstd4(mv_all, tag):
                """Batched over all 4 t: one Sqrt instruction (table-load
                friendly: its deps force it after the last t's stats)."""
                sd4 = p3s.tile([P, NQT, 1], FP, tag="sd4", name=f"sd4{tag}")
                nc.scalar.activation(out=sd4, in_=mv_all[:, :, 1:2], func=AF.Sqrt,
                                     bias=eps_sb[:, 0:1])
                rstd4 = p3s.tile([P, NQT, 1], FP, tag="rstd4", name=f"rstd4{tag}")
                nc.vector.reciprocal(out=rstd4, in_=sd4)
                return rstd4

            # half 0 of r1/stats was computed mid-stream (r1_half(0))
            mv1 = p3s.tile([P, NQT, 2], FP, tag="mv1", name="mv1")
            for t in range(NQT):
                nc.vector.tensor_add(r1s[t][:, 512:1024], Qp[t][:, 512:1024],
                                     Ob[:, t, 512:1024])
                nc.vector.bn_stats(out=st1[t][:, 1, :], in_=r1s[t][:, 512:1024])
                nc.vector.bn_aggr(out=mv1[:, t, :], in_=st1[t])
            rstd1 = ln_rstd4(mv1, "a")
            for t in range(NQT):
                nc.vector.tensor_scalar(
                    out=O1[t], in0=r1s[t], scalar1=mv1[:, t, 0:1],
                    scalar2=rstd1[:, t], op0=ALU.subtract, op1=ALU.mult)
                tp = tpp3.tile([P, NDT, P], BF, tag="tp3", name=f"tp3_{t}")
                for i in range(NDT):
                    nc.tensor.matmul(tp[:, i, :], O1[t][:, P * i:P * (i + 1)],
                                     identb, is_transpose=True,
                                     start=(i == 0), stop=(i == NDT - 1))
                nc.vector.tensor_copy(OTb[:, :, P * t:P * (t + 1)], tp)

            r2s, st2 = [], []
            for t in range(NQT):
                r2 = p3s.tile([P, DIM], BF, tag="r2", name=f"r2_{t}", bufs=4)
                s2 = p3s.tile([P, 2, 6], FP, tag="st2", name=f"st2_{t}", bufs=4)
                for c in range(2):
                    ps = fps.tile([P, 512], FP, tag="fps", name=f"fps{t}_{c}")
                    for i in range(NDT):
                        nc.tensor.matmul(ps, OTb[:, i, P * t:P * (t + 1)],
                                         wo_sb[:, i, 512 * c:512 * (c + 1)],
                                         start=(i == 0), stop=(i == NDT - 1))
                    g = p3s.tile([P, 512], BF, tag="g", name=f"g{t}_{c}", bufs=4)
                    nc.scalar.activation(out=g, in_=ps, func=AF.Gelu)
                    nc.vector.tensor_add(r2[:, 512 * c:512 * (c + 1)],
                                         O1[t][:, 512 * c:512 * (c + 1)], g)
                    nc.vector.bn_stats(out=s2[:, c, :],
                                       in_=r2[:, 512 * c:512 * (c + 1)])
                r2s.append(r2)
                st2.append(s2)

            mv2 = p3s.tile([P, NQT, 2], FP, tag="mv2", name="mv2")
            for t in range(NQT):
                nc.vector.bn_aggr(out=mv2[:, t, :], in_=st2[t])
            rstd2 = ln_rstd4(mv2, "b")
            nc.vector.tensor_mul(rstd2, rstd2,
                                 qm.rearrange("p (t o) -> p t o", o=1))
            for t in range(NQT):
                fin = p3s.tile([P, DIM], BF, tag="fin", name=f"fin_{t}", bufs=4)
                nc.vector.tensor_scalar(
                    out=fin, in0=r2s[t], scalar1=mv2[:, t, 0:1],
                    scalar2=rstd2[:, t], op0=ALU.subtract, op1=ALU.mult)
                nc.sync.dma_start(out=out[P * t:P * (t + 1), :], in_=fin)

    nc.compile()
    return nc


def _get_nc():
    global _CACHED_NC
    if _CACHED_NC is None:
        _CACHED_NC = build_nc()
    return _CACHED_NC


E4NP = ml_dtypes.float8_e4m3
BFNP = ml_dtypes.bfloat16


def _g_pack(m):
    """[1024 (din), cols] -> [128, 4, 2, cols] DoubleRow group layout."""
    cols = m.shape[1]
    return np.ascontiguousarray(
        m.reshape(NG, 2, P, cols).transpose(2, 0, 1, 3))


def _j_pack(m):
    """[8*128 rows, cols] -> [128, 8, cols] row-tile layout."""
    cols = m.shape[1]
    return np.ascontiguousarray(
        m.reshape(NDT, P, cols).transpose(1, 0, 2))


def _make_in_maps(inputs):
    Q, K, V = inputs["Q"], inputs["K"], inputs["V"]
    mask_Q, mask_K = inputs["mask_Q"], inputs["mask_K"]
    wqT = np.ascontiguousarray(inputs["Wq"].T.astype(np.float32))
    wkT = np.ascontiguousarray(inputs["Wk"].T.astype(np.float32))
    wvT = np.ascontiguousarray(inputs["Wv"].T.astype(np.float32))
    woT = np.ascontiguousarray(inputs["Wo"].T.astype(np.float32))

    wq8 = _g_pack(wqT * 16.0).astype(E4NP)
    wk8 = _g_pack(wkT * 16.0).astype(E4NP)
    wv8 = _g_pack(wvT).astype(E4NP)
    wqb = _j_pack(wqT).astype(BFNP)
    wob = _j_pack(woT).astype(BFNP)

    in_maps = []
    for c in range(8):
        b, q0 = c // 2, (c % 2) * NQ
        kT = np.ascontiguousarray(K[b].T)
        vT = np.ascontiguousarray(V[b].T)
        qT = np.ascontiguousarray(Q[b, q0:q0 + NQ, :].T)
        km01 = np.where(mask_K[b], 0.0, 1.0).astype(np.float32)
        qm01 = np.where(mask_Q[b, q0:q0 + NQ], 0.0, 1.0).astype(np.float32)
        maskd = np.concatenate([km01.reshape(NKT, P).T,
                                qm01.reshape(NQT, P).T], axis=1)
        in_maps.append({
            "qt8": _g_pack(qT).astype(E4NP),
            "wq8": wq8,
            "kt8": _g_pack(kT).astype(E4NP),
            "wk8": wk8,
            "wv8": wv8,
            "vt8": _g_pack(vT).astype(E4NP),
            "qtb": _j_pack(qT).astype(BFNP),
            "wqb": wqb,
            "wo": wob,
            "maskd": np.ascontiguousarray(maskd),
        })
    return in_maps


def _assemble(results):
    out = np.empty((B, 1024, DIM), np.float32)
    for c in range(8):
        b, q0 = c // 2, (c % 2) * NQ
        out[b, q0:q0 + NQ, :] = results[c]["out"].astype(np.float32)
    return out


def kernel(**inputs):
    nc = _get_nc()
    res = run_bass_kernel_spmd(nc, _make_in_maps(inputs), core_ids=list(range(8)))
    return _assemble(res.results)


def kernel_profiled(inputs, **kw):
    nc = _get_nc()
    res = run_bass_kernel_spmd(nc, _make_in_maps(inputs),
                               core_ids=list(range(8)), trace=True, **kw)
    return _assemble(res.results), res
